# revision 42
# baseline (speedup 1.0000x reference)
"""GroupedSwiGLU MoE kernel for 8x Trainium2 NeuronCores.

Strategy: load-balanced expert-parallel. The SPMD constraint requires one
program for all cores, so each core runs a fixed list of token *segments*
(e.g. 384+384+256 = 1024 tokens), each segment expert-pure with its own
full weight-set input. The host solves an exact packing of the ragged
per-expert token counts into the 8x per-core slot structure (zero padding
for the reference counts), so every core does 1/8 of the FLOPs instead of
padding to the largest expert.

Inside each core, per segment:
  phase 1: gateT/upT[inter, tok] = Wg/Wu^T-contracted matmuls vs xT
  swiglu : hT = silu(gateT) * upT
  phase 2: out[tok, hid] = hT^T-contracted matmuls vs Wd, scaled by probs
All matmul operands bf16 (fp32 PSUM accumulate); host does the
transpose/tiling/padding and the final gather.
"""

import numpy as np
import ml_dtypes
from contextlib import ExitStack

import concourse.bass as bass
import concourse.mybir as mybir
import concourse.tile as tile
from concourse.bacc import Bacc
from concourse.bass_utils import run_bass_kernel_spmd

E = 8
HID = 2048
INTER = 1408
P = 128
KO_H = HID // P    # 16 k-tiles for phase-1 contraction
KO_I = INTER // P  # 11 k-tiles for phase-2 contraction / m-tiles in phase 1
NF = 512           # phase-2 moving free chunk (hid)

F32 = mybir.dt.float32
BF16 = mybir.dt.bfloat16
NP_BF16 = ml_dtypes.bfloat16

_nc_cache: dict = {}


def _build(segs: tuple):
    """Per-core Bass program for segment sizes `segs` (each a multiple of
    128). Segment i contracts against its own weight set wg{i}/wu{i}/wd{i}."""
    nc = Bacc()
    T = sum(segs)
    n_t = T // P
    n_nf = HID // NF

    xTs = [nc.dram_tensor(f"xT{s}", [P, KO_H, n], BF16, kind="ExternalInput")
           for s, n in enumerate(segs)]
    wgs = [nc.dram_tensor(f"wg{s}", [P, KO_I, KO_H, P], BF16, kind="ExternalInput")
           for s in range(len(segs))]
    wus = [nc.dram_tensor(f"wu{s}", [P, KO_I, KO_H, P], BF16, kind="ExternalInput")
           for s in range(len(segs))]
    wds = [nc.dram_tensor(f"wd{s}", [P, KO_I, HID], BF16, kind="ExternalInput")
           for s in range(len(segs))]
    probs = nc.dram_tensor("probs", [P, n_t], F32, kind="ExternalInput")
    out = nc.dram_tensor("out", [T, HID], BF16, kind="ExternalOutput")

    # DMA queue assignment: three independent DGE rings so the two weight
    # streams and the bulk x/wd/out traffic don't round-robin against each
    # other mid-phase.  wg -> SP (sync), wu -> SWDGE (gpsimd),
    # xT/probs/wd -> ACT (scalar), out -> SP (fills sync's idle ph2 window).
    with tile.TileContext(nc) as tc, ExitStack() as ctx:
        # Pool depths shrink for oversized fallback schemes (>1024 tokens)
        # so the resident tiles still fit in SBUF.
        big = T > 1024
        resident = ctx.enter_context(tc.tile_pool(name="resident", bufs=1))
        wpool = ctx.enter_context(tc.tile_pool(name="weights", bufs=2 if big else 3))
        wdpool = ctx.enter_context(tc.tile_pool(name="wdown", bufs=1 if big else 2))
        tmp = ctx.enter_context(tc.tile_pool(name="tmp", bufs=2 if big else 3))
        opool = ctx.enter_context(tc.tile_pool(name="outp", bufs=2 if big else 3))
        psum = ctx.enter_context(tc.tile_pool(name="psum", bufs=3, space="PSUM"))
        psum2 = ctx.enter_context(tc.tile_pool(name="psum2", bufs=2, space="PSUM"))

        # PE warm-up: the HAM clock gate runs the first ~3.4us of matmuls at
        # 1.2 GHz.  Issue dependency-free garbage matmuls at t=0 so the PE is
        # at 2.4 GHz by the time the first real operands land, and the clock
        # ramp overlaps the DMA head instead of the real work.
        warm = resident.tile([P, 640], BF16)
        nc.vector.memset(warm[:], 0.0)
        pw = psum2.tile([P, NF], F32, tag="po")
        for _ in range(8):
            nc.tensor.matmul(pw[:], warm[:, 0:P], warm[:, P:], start=True, stop=True)

        # DMA trigger budget matters: each dma_start costs ~600ns of issuing-
        # sequencer time (DIRECT2D), so bulk tensors use ONE contiguous
        # trigger each, and the ACT ring (which also runs the silu ops)
        # carries only the 3 wd triggers.  First-matmul deps (wg0 m0 on SP,
        # xT0 + wu0 m0 on SWDGE) are issued first on their rings.
        preload = {}
        wg_m0 = wpool.tile([P, KO_H, P], BF16, tag="wg")
        nc.scalar.dma_start(wg_m0[:], wgs[0][:, 0])

        # xT0 arrives in 4 k-chunks on the fast HWDGE sync ring so the m0
        # matmul pipeline starts on the first 0.4MB instead of waiting for
        # the whole segment load (SWDGE's per-descriptor cost is too high
        # for the latency-critical head).
        xT_sbs = []
        xT_sbs.append(resident.tile([P, KO_H, segs[0]], BF16, tag="xT0", name="xT0_sb"))
        for kc in range(0, KO_H, 4):
            nc.sync.dma_start(xT_sbs[0][:, kc : kc + 4], xTs[0][:, kc : kc + 4])
        wu_m0 = wpool.tile([P, KO_H, P], BF16, tag="wu")
        nc.gpsimd.dma_start(wu_m0[:], wus[0][:, 0])
        preload[(0, 0)] = (wg_m0, wu_m0)
        # m1 weights go ahead of the remaining xT segments in the SWDGE FIFO
        # so the m1 matmuls aren't blocked behind 2.7MB of token loads.
        wg_m1 = wpool.tile([P, KO_H, P], BF16, tag="wg")
        nc.scalar.dma_start(wg_m1[:], wgs[0][:, 1])
        wu_m1 = wpool.tile([P, KO_H, P], BF16, tag="wu")
        nc.gpsimd.dma_start(wu_m1[:], wus[0][:, 1])
        preload[(0, 1)] = (wg_m1, wu_m1)
        # Later segments' tokens ride the ACT ring; their triggers are
        # emitted mid-seg0 (below) so the early DMA window carries only
        # xT0 + the seg0 weight streams.
        for s in range(1, len(segs)):
            xT_sbs.append(resident.tile([P, KO_H, segs[s]], BF16, tag=f"xT{s}", name=f"xT{s}_sb"))

        probs_dma = resident.tile([P, n_t], F32)
        nc.sync.dma_start(probs_dma[:], probs[:])
        # Bounce through DVE so phase-2 scaling (DVE) only ever needs the PE
        # wait: the TensorScalar ISA slot can't carry a second (DMA) wait.
        probs_sb = resident.tile([P, n_t], F32)
        nc.vector.tensor_copy(probs_sb[:], probs_dma[:])
        hT_sb = resident.tile([P, KO_I, T], BF16)

        seg_offs = []
        o = 0
        for n in segs:
            seg_offs.append(o)
            o += n
        S = len(segs)
        wd_tiles = [None] * S

        # Anchor map: (seg, m) -> deferred bulk loads emitted after that
        # m-tile's silu, with a tiny WAW copy from the silu output forcing
        # the DMA to actually wait (the scheduler hoists dependency-free
        # triggers).  wd0 is spread as 11 thin k-slices through the ph1
        # windows' spare bandwidth; later wd sets land whole in the ph2
        # valleys.  ("wd", j, k) = k-slice, ("wdw", j) = whole set.
        anchors = {}
        if S == 1:
            for i in range(KO_I):
                anchors.setdefault((0, min(2 + i, KO_I - 2)), []).append(("wd", 0, i))
        else:
            for i in range(KO_I):
                if i < 8:
                    anchors.setdefault((0, 2 + i), []).append(("wd", 0, i))
                else:
                    anchors.setdefault((1, i - 8), []).append(("wd", 0, i))
            for j in range(1, S):
                m_at = KO_I - 1 - (S - 1 - j)
                anchors.setdefault((S - 1, max(3, m_at)), []).append(("wdw", j))
            for j in range(1, S):
                anchors.setdefault((j - 1, 5), []).append(("xT", j))

        def emit_ph1(s):
            ntok = segs[s]
            off = seg_offs[s]
            xT_sb = xT_sbs[s]
            for m in range(KO_I):
                if (s, m) in preload:
                    wg_m, wu_m = preload[(s, m)]
                else:
                    wg_m = wpool.tile([P, KO_H, P], BF16, tag="wg", name="wg_m")
                    nc.sync.dma_start(wg_m[:], wgs[s][:, m])
                    wu_m = wpool.tile([P, KO_H, P], BF16, tag="wu", name="wu_m")
                    nc.gpsimd.dma_start(wu_m[:], wus[s][:, m])
                pg = psum.tile([P, ntok], F32, tag="pg", name="pg")
                pu = psum.tile([P, ntok], F32, tag="pu", name="pu")
                for k in range(KO_H):
                    nc.tensor.matmul(
                        pg[:], wg_m[:, k], xT_sb[:, k],
                        start=(k == 0), stop=(k == KO_H - 1),
                    )
                for k in range(KO_H):
                    nc.tensor.matmul(
                        pu[:], wu_m[:, k], xT_sb[:, k],
                        start=(k == 0), stop=(k == KO_H - 1),
                    )
                sg = tmp.tile([P, ntok], F32, tag="sg", name="sg")
                nc.scalar.activation(
                    sg[:], pg[:], mybir.ActivationFunctionType.Silu
                )
                # ACT copy of up-psum so the DVE mul has a single-engine wait
                su = tmp.tile([P, ntok], F32, tag="su", name="su")
                nc.scalar.copy(su[:], pu[:])
                nc.vector.tensor_mul(
                    hT_sb[:, m, off : off + ntok], sg[:], su[:]
                )
                for act in anchors.get((s, m), ()):
                    if act[0] == "wd":
                        _, j, ks = act
                        if wd_tiles[j] is None:
                            wd_tiles[j] = wdpool.tile(
                                [P, KO_I, HID], BF16, tag="wd", name=f"wd_sb{j}"
                            )
                        nc.vector.tensor_copy(
                            wd_tiles[j][0:1, ks, 0:2], sg[0:1, 0:2]
                        )
                        nc.scalar.dma_start(wd_tiles[j][:, ks], wds[j][:, ks])
                    elif act[0] == "wdw":
                        j = act[1]
                        wd_tiles[j] = wdpool.tile(
                            [P, KO_I, HID], BF16, tag="wd", name=f"wd_sb{j}"
                        )
                        nc.vector.tensor_copy(
                            wd_tiles[j][0:1, 0, 0:2], sg[0:1, 0:2]
                        )
                        nc.scalar.dma_start(wd_tiles[j][:], wds[j][:])
                    else:
                        j = act[1]
                        nc.vector.tensor_copy(
                            xT_sbs[j][0:1, 0, 0:2], sg[0:1, 0:2]
                        )
                        nc.scalar.dma_start(xT_sbs[j][:], xTs[j][:])

        def emit_ph2(s):
            ntok = segs[s]
            off = seg_offs[s]
            wd_sb = wd_tiles[s]
            # Phase 2: out tiles [128 tok, 512 hid], contract over inter.
            # The 4 hid-chunks of one token row stage into one [128, 2048]
            # tile and leave in a single DMA (one trigger per token tile).
            for t in range(ntok // P):
                tg = off // P + t
                ot = opool.tile([P, HID], BF16, tag="ot", name="ot")
                for n in range(n_nf):
                    po = psum2.tile([P, NF], F32, tag="po", name="po")
                    for k in range(KO_I):
                        nc.tensor.matmul(
                            po[:], hT_sb[:, k, off + t * P : off + (t + 1) * P],
                            wd_sb[:, k, bass.ts(n, NF)],
                            start=(k == 0), stop=(k == KO_I - 1),
                        )
                    nc.vector.tensor_scalar_mul(
                        ot[:, bass.ts(n, NF)], po[:], probs_sb[:, tg : tg + 1]
                    )
                nc.sync.dma_start(out[bass.ts(tg, P)], ot[:])

        # All ph1 phases run back-to-back (the weight stream never pauses,
        # and segment boundaries pipeline like interior m-tiles), then all
        # ph2 phases: their windows need almost no DMA, forming the valleys
        # where the wd sets and staged outputs transfer.
        for s in range(S):
            emit_ph1(s)
        for s in range(S):
            emit_ph2(s)
    nc.finalize()
    return nc


# ---------------------------------------------------------------------------
# Host-side packing


def _pack_segments(counts):
    """Pack ragged per-expert token counts into a uniform per-core slot
    structure. Returns (scheme, assignment) where scheme is the per-core
    tuple of slot sizes and assignment[core] is a list of
    (expert, start_within_expert, n_real) per slot — or None if no listed
    scheme fits."""
    counts = [int(c) for c in counts]
    n_e = len(counts)

    schemes = [
        (384, 384, 256),
        (512, 384, 128),
        (512, 512, 128),
        (512, 384, 256),
        (384, 384, 384),
        (512, 512, 256),
        (512, 512, 384),
        (512, 512, 512),
        (512, 512, 512, 128),
        (512, 512, 512, 256),
        (512, 512, 512, 384),
        (512, 512, 512, 512),
    ]
    for scheme in schemes:
        sizes = sorted(set(scheme), reverse=True)
        cap = {sz: 8 * scheme.count(sz) for sz in sizes}

        # DFS over experts: choose per-expert usage of each slot class.
        def options(c):
            """All (usage-vector, waste) covering c, waste < min slot size."""
            opts = []
            maxn = [min(cap[sz], (c + sz - 1) // sz) for sz in sizes]

            def rec(i, rem, used):
                if rem <= 0:
                    opts.append(tuple(used) + (0,) * (len(sizes) - len(used)))
                    return
                if i == len(sizes):
                    return
                for n in range(maxn[i] + 1):
                    # a part must be "useful": last class can overshoot by
                    # less than one slot
                    used.append(n)
                    rec(i + 1, rem - n * sizes[i], used)
                    used.pop()

            rec(0, c, [])
            # keep non-dominated, low-waste options
            opts = sorted(
                set(opts),
                key=lambda u: (sum(x * sz for x, sz in zip(u, sizes)), sum(u)),
            )
            return opts[:24]

        expert_opts = [options(c) for c in counts]
        if any(not o for o in expert_opts):
            continue

        sol = [None] * n_e

        def dfs(e, caps):
            if e == n_e:
                return True
            for u in expert_opts[e]:
                if all(u[i] <= caps[i] for i in range(len(sizes))):
                    sol[e] = u
                    if dfs(e + 1, [caps[i] - u[i] for i in range(len(sizes))]):
                        return True
            sol[e] = None
            return False

        if not dfs(0, [cap[sz] for sz in sizes]):
            continue

        # Materialize parts per class, splitting each expert contiguously.
        parts = {sz: [] for sz in sizes}
        for e in range(n_e):
            pos = 0
            rem = counts[e]
            for i, sz in enumerate(sizes):
                for _ in range(sol[e][i]):
                    n_real = min(rem, sz)
                    if n_real > 0:
                        parts[sz].append((e, pos, n_real))
                        pos += n_real
                        rem -= n_real
                    else:
                        parts[sz].append((e, pos, 0))
        # Assign: core c takes the c-th part of each slot in scheme order.
        assignment = []
        idx = {sz: 0 for sz in sizes}
        ok = True
        for c in range(8):
            slots = []
            for sz in scheme:
                lst = parts[sz]
                i = idx[sz]
                if i < len(lst):
                    slots.append(lst[i])
                else:
                    slots.append((0, 0, 0))  # fully padded slot
                idx[sz] += 1
            assignment.append(slots)
        for sz in sizes:
            if idx[sz] < len(parts[sz]):
                ok = False  # parts left over: scheme infeasible
        if ok:
            return scheme, assignment
    return None, None


def _pack_x(x_pad, T):
    # xT[p, k, t] = x_pad[t, k*128+p]
    return np.ascontiguousarray(
        x_pad.T.reshape(KO_H, P, T).transpose(1, 0, 2)
    ).astype(NP_BF16)


_wcache: dict = {}


def _pack_weights(wg_e, wu_e, wd_e, key):
    if key in _wcache:
        return _wcache[key]
    # wg[p, m, k, i] = w_gate[k*128+p, m*128+i]
    wgt = np.ascontiguousarray(
        wg_e.reshape(KO_H, P, KO_I, P).transpose(1, 2, 0, 3)
    ).astype(NP_BF16)
    wut = np.ascontiguousarray(
        wu_e.reshape(KO_H, P, KO_I, P).transpose(1, 2, 0, 3)
    ).astype(NP_BF16)
    # wd[p, k, h] = w_down[k*128+p, h]
    wdt = np.ascontiguousarray(
        wd_e.reshape(KO_I, P, HID).transpose(1, 0, 2)
    ).astype(NP_BF16)
    _wcache[key] = (wgt, wut, wdt)
    return _wcache[key]


def _run(inputs, trace=False):
    x = np.asarray(inputs["permuted_x"], np.float32)
    probs = np.asarray(inputs["permuted_probs"], np.float32)
    wg = np.asarray(inputs["w_gate"], np.float32)
    wu = np.asarray(inputs["w_up"], np.float32)
    wd = np.asarray(inputs["w_down"], np.float32)
    counts = np.asarray(inputs["tokens_per_expert"]).astype(np.int64)
    offs = np.concatenate([[0], np.cumsum(counts)])
    assert offs[-1] == x.shape[0]

    _wcache.clear()

    scheme, assignment = _pack_segments(counts)
    if scheme is None:
        # Fallback: pad every core to the largest expert (always feasible),
        # split into 512-token segments all holding the same expert (free
        # dims and PSUM tiles must stay <= 512).
        T = int(max(1, counts.max()))
        T = ((T + 511) // 512) * 512
        scheme = (512,) * (T // 512)
        assignment = [
            [
                (e, i * 512, max(0, min(512, int(counts[e]) - i * 512)))
                for i in range(T // 512)
            ]
            for e in range(E)
        ]

    key = tuple(scheme)
    if key not in _nc_cache:
        _nc_cache[key] = _build(key)
    nc = _nc_cache[key]

    T = sum(scheme)
    in_maps = []
    for c in range(E):
        p_pad = np.zeros((T,), np.float32)
        im = {}
        off = 0
        for si, (sz, (e, pos, n)) in enumerate(zip(scheme, assignment[c])):
            g0 = int(offs[e]) + pos
            x_pad = np.zeros((sz, HID), np.float32)
            if n > 0:
                x_pad[:n] = x[g0 : g0 + n]
                p_pad[off : off + n] = probs[g0 : g0 + n]
            im[f"xT{si}"] = _pack_x(x_pad, sz)
            wgt, wut, wdt = _pack_weights(wg[e], wu[e], wd[e], e)
            im[f"wg{si}"] = wgt
            im[f"wu{si}"] = wut
            im[f"wd{si}"] = wdt
            off += sz
        im["probs"] = np.ascontiguousarray(
            p_pad.reshape(T // P, P).T
        ).astype(np.float32)
        in_maps.append(im)

    res = run_bass_kernel_spmd(nc, in_maps, core_ids=list(range(E)), trace=trace)

    y = np.empty((x.shape[0], HID), np.float32)
    for c in range(E):
        out_c = np.asarray(res.results[c]["out"], dtype=np.float32)
        off = 0
        for sz, (e, pos, n) in zip(scheme, assignment[c]):
            if n > 0:
                g0 = int(offs[e]) + pos
                y[g0 : g0 + n] = out_c[off : off + n]
            off += sz
    return y, res


def kernel(**inputs) -> np.ndarray:
    y, _ = _run(inputs, trace=False)
    return y
